# revision 7
# baseline (speedup 1.0000x reference)
"""Trainium2 Bass kernel for nn_MCM_37031208026850.

Strategy (8 NeuronCores, SPMD):
  - Shard the four 4096x512x4096 score GEMMs by query row: core r owns global
    rows [512r, 512(r+1)) (= batch b=r//2, image half r%2).
  - Each core projects its q-slices (cq, tq) and k-slices (ck, tk) locally,
    AllGathers the k projections (512x512 fp32 per core -> 512x4096 full),
    then streams k in 512-column chunks through the PE with q stationary.
  - The mean-over-HW term never touches the score matrix: mean = q @ ksum
    with ksum precomputed on host. Only the max needs the full scores; the
    vector engine max-reduces each PSUM tile as it is produced.
  - Tiny AllGather of the pooled co vectors (4x512 per core); every core
    computes all 16 softmax gates and selects/broadcasts the two gate images
    it needs with host-supplied one-hot matmuls.
  - The 1x1 value convs are folded on host (W512_64 @ Wv_c), computed
    full-batch per core; gating and the three 3x3 fusion convs run
    full-image per core (pair-duplicated), host takes core 2b's output.
  - Score/projection/conv matmuls run in float32r (1 cycle/row on TRN2,
    ~1.4e-4 rel err vs 2.5e-3 for bf16).
"""
import sys
sys.path.insert(0, "/opt/trn_rl_repo")

import numpy as np

import concourse.bass as bass
import concourse.mybir as mybir
import concourse.tile as tile
from concourse import bacc
from concourse import bass_utils
from concourse.masks import make_identity

B, C, H, W = 4, 512, 32, 32
HW = H * W
SCALE = 1.0 / C ** 0.5
NCORES = 8
P = 128
KT = C // P          # 4 k-tiles over channels
S = 512              # q-rows per core
NCH = 8              # global column chunks of 512
F32 = mybir.dt.float32
F32R = mybir.dt.float32r
AX = mybir.AxisListType.X
AF = mybir.ActivationFunctionType
MUL = mybir.AluOpType.mult
ADD = mybir.AluOpType.add


# ----------------------------------------------------------------------------
# host-side preparation
# ----------------------------------------------------------------------------

def host_prep(inputs):
    """Build the 8 per-core input maps from the full problem inputs."""
    xc = np.ascontiguousarray(inputs["xc"], dtype=np.float32)
    xt = np.ascontiguousarray(inputs["xt"], dtype=np.float32)
    f = lambda k: np.ascontiguousarray(inputs[k], dtype=np.float32)
    Wq_c, bq_c = f("Wq_c"), f("bq_c")
    Wk_c, bk_c = f("Wk_c"), f("bk_c")
    Wv_c, bv_c = f("Wv_c"), f("bv_c")
    Wq_t, bq_t = f("Wq_t"), f("bq_t")
    Wk_t, bk_t = f("Wk_t"), f("bk_t")
    W64, b64 = f("W512_64"), f("b512_64")
    W1, b1 = f("W1"), f("b1")
    W2, b2 = f("W2"), f("b2")
    W3, b3 = f("W3"), f("b3")

    xcG = np.ascontiguousarray(
        xc.reshape(B, C, HW).transpose(1, 0, 2).reshape(C, B * HW))
    xtT = np.ascontiguousarray(
        xt.transpose(2, 0, 1).reshape(C, B * HW))

    # ksum[c, kk*4+b]: column-sums of the k matrices per batch, /HW (mean),
    # computed from input sums so the score matrix is never needed.
    xc_sum = xc.sum(axis=(2, 3))                      # (B, C)
    xt_sum = xt.sum(axis=1)                           # (B, C)
    ks_ck = (Wk_c @ xc_sum.T + HW * bk_c[:, None]) / HW    # (C, B)
    ks_tk = (Wk_t @ xt_sum.T + HW * bk_t[:, None]) / HW    # (C, B)
    ksums = np.concatenate([ks_ck, ks_tk], axis=1)    # (C, 8) [kk*4+b]

    Wcv = W64 @ Wv_c                                  # (64, C)
    bcv = W64 @ bv_c                                  # (64,)
    wcv64 = np.ascontiguousarray(np.concatenate([Wcv, Wcv], axis=0).T)   # (C,128)
    wtv64 = np.ascontiguousarray(np.concatenate([W64, W64], axis=0).T)   # (C,128)
    bcv64 = np.concatenate([bcv, bcv]).reshape(P, 1)
    b64dup = np.concatenate([b64, b64]).reshape(P, 1)

    def kmaj(a):
        # (C, n) -> (P, KT*n) k-major layout matching SBUF tiles
        n = a.shape[1]
        return a.reshape(KT, P, n).transpose(1, 0, 2).reshape(P, KT * n)

    # blob_k: k-projection weights (needed first)
    blob_k = np.hstack([kmaj(Wk_c.T), kmaj(Wk_t.T)])                  # (P, 4096)
    # blob_q: q-projection weights
    blob_q = np.hstack([kmaj(Wq_c.T), kmaj(Wq_t.T)])                  # (P, 4096)
    w3t_full = W3.transpose(1, 2, 3, 0).reshape(P, 9 * 64)
    pad = np.zeros((P - 64, 9 * 64), np.float32)
    # blob_v: value weights + ksums + sels + conv weights
    sel_pad = np.zeros((P, 2 * P), np.float32)   # filled per-core later
    blob_v_shared = np.hstack([
        kmaj(wcv64), kmaj(wtv64), kmaj(ksums),
        W1.transpose(1, 2, 3, 0).reshape(P, 9 * 64),
        W2.transpose(1, 2, 3, 0).reshape(P, 9 * 64),
        np.vstack([w3t_full[:64], pad]),
        np.vstack([w3t_full[64:], pad]),
    ])                                                                # (P, ...)
    # blob_b (fp32): all biases column-packed
    cpad = np.zeros((P - 64, 1), np.float32)
    blob_b = np.hstack([
        bq_c.reshape(KT, P).T, bk_c.reshape(KT, P).T,
        bq_t.reshape(KT, P).T, bk_t.reshape(KT, P).T,
        bcv64, b64dup,
        np.vstack([b1.reshape(64, 1), cpad]),
        np.vstack([b2.reshape(64, 1), cpad]),
        np.vstack([b3.reshape(64, 1), cpad]),
    ]).astype(np.float32)                                             # (P, 19)
    shared = {
        "blob_k": np.ascontiguousarray(blob_k, dtype=np.float32),
        "blob_q": np.ascontiguousarray(blob_q, dtype=np.float32),
        "blob_b": np.ascontiguousarray(blob_b, dtype=np.float32),
    }

    in_maps = []
    for r in range(NCORES):
        myb = r // 2
        cols = slice(S * r, S * (r + 1))
        bcols = slice(HW * myb, HW * (myb + 1))
        sel1 = np.zeros((16, P), np.float32)   # T1 = [c_co; ct_co] combos 0,1
        sel2 = np.zeros((16, P), np.float32)   # T2 = [t_co; tc_co] combos 3,2
        for p in range(P):
            sel1[(0 if p < 64 else 1) * 4 + myb, p] = 1.0
            sel2[(3 if p < 64 else 2) * 4 + myb, p] = 1.0
        m = dict(shared)
        m["xcq"] = np.ascontiguousarray(xcG[:, cols])
        m["xtq"] = np.ascontiguousarray(xtT[:, cols])
        m["xcb"] = np.ascontiguousarray(xcG[:, bcols])
        m["xtb"] = np.ascontiguousarray(xtT[:, bcols])
        m["blob_v"] = np.ascontiguousarray(
            np.hstack([blob_v_shared,
                       np.vstack([sel1, np.zeros((P - 16, P), np.float32)]),
                       np.vstack([sel2, np.zeros((P - 16, P), np.float32)])]),
            dtype=np.float32)
        in_maps.append(m)
    return in_maps


# ----------------------------------------------------------------------------
# device program
# ----------------------------------------------------------------------------

def build_program(time_reps: int = 1, debug: bool = False):
    """Build + bacc-compile the SPMD Bass program.

    time_reps > 1 wraps the three compute segments in For_i loops (collectives
    stay outside) so wall-clock deltas between different reps counts measure
    pure per-iteration compute time.
    """
    import contextlib
    nc = bacc.Bacc("TRN2", target_bir_lowering=False, debug=False,
                   num_devices=NCORES)

    def din(name, shape, dtype=F32R):
        return nc.dram_tensor(name, list(shape), dtype, kind="ExternalInput")

    xcq_d = din("xcq", (C, S)); xtq_d = din("xtq", (C, S))
    xcb_d = din("xcb", (C, HW)); xtb_d = din("xtb", (C, HW))
    blob_k_d = din("blob_k", (P, 2 * KT * 512))
    blob_q_d = din("blob_q", (P, 2 * KT * 512))
    VBLOB = 512 + 512 + 32 + 576 + 576 + 576 + 576 + 128 + 128
    blob_v_d = din("blob_v", (P, VBLOB))
    blob_b_d = din("blob_b", (P, 21), F32)

    outp_d = nc.dram_tensor("outp", [64, HW], F32, kind="ExternalOutput")
    if debug:
        dbg_co_d = nc.dram_tensor("dbg_co", [4, S], F32, kind="ExternalOutput")
        dbg_gates_d = nc.dram_tensor("dbg_gates", [16, HW], F32,
                                     kind="ExternalOutput")
        dbg_cv_d = nc.dram_tensor("dbg_cv", [P, HW], F32, kind="ExternalOutput")
        dbg_cq_d = nc.dram_tensor("dbg_cq", [P, KT, S], F32,
                                  kind="ExternalOutput")

    KMAJ = "(kt p) n -> p kt n"

    with tile.TileContext(nc) as tc:
      with tc.tile_pool(name="consts", bufs=1) as cons, \
           tc.tile_pool(name="dram", bufs=1, space="DRAM") as dram:
        # ---------------- static loads (phase-ordered) ----------------
        xcq_sb = cons.tile([P, KT, S], F32R)
        xtq_sb = cons.tile([P, KT, S], F32R)
        nc.sync.dma_start(xcq_sb, xcq_d.ap().rearrange(KMAJ, p=P))
        nc.sync.dma_start(xtq_sb, xtq_d.ap().rearrange(KMAJ, p=P))
        kblob_sb = cons.tile([P, 2, KT, 512], F32R)
        nc.sync.dma_start(
            kblob_sb, blob_k_d.ap().rearrange("p (k kt n) -> p k kt n",
                                              k=2, kt=KT))
        qblob_sb = cons.tile([P, 2, KT, 512], F32R)
        nc.sync.dma_start(
            qblob_sb, blob_q_d.ap().rearrange("p (k kt n) -> p k kt n",
                                              k=2, kt=KT))
        vblob_sb = cons.tile([P, VBLOB], F32R)
        nc.sync.dma_start(vblob_sb, blob_v_d.ap())
        bblob_sb = cons.tile([P, 21], F32)
        nc.sync.dma_start(bblob_sb, blob_b_d.ap())
        xcb_sb = cons.tile([P, KT, HW], F32R)
        xtb_sb = cons.tile([P, KT, HW], F32R)
        nc.sync.dma_start(xcb_sb, xcb_d.ap().rearrange(KMAJ, p=P))
        nc.sync.dma_start(xtb_sb, xtb_d.ap().rearrange(KMAJ, p=P))

        # blob views
        def vsl(lo, n):
            return vblob_sb[:, lo:lo + n]

        wcv_sb = vsl(0, 512).rearrange("p (kt n) -> p kt n", kt=KT)
        wtv_sb = vsl(512, 512).rearrange("p (kt n) -> p kt n", kt=KT)
        ksums_sb = vsl(1024, 32).rearrange("p (kt n) -> p kt n", kt=KT)
        w1t_sb = vsl(1056, 576).rearrange("p (t n) -> p t n", t=9)
        w2t_sb = vsl(1632, 576).rearrange("p (t n) -> p t n", t=9)
        w3a_sb = vblob_sb[0:64, 2208:2784].rearrange("p (t n) -> p t n", t=9)
        w3b_sb = vblob_sb[0:64, 2784:3360].rearrange("p (t n) -> p t n", t=9)
        sel1_sb = vblob_sb[0:16, 3360:3488]
        sel2_sb = vblob_sb[0:16, 3488:3616]
        conv_w = [w1t_sb, w2t_sb]
        bq_sb = [bblob_sb[:, 0:4], bblob_sb[:, 8:12]]
        bk_sb = [bblob_sb[:, 4:8], bblob_sb[:, 12:16]]
        bcv_sb = bblob_sb[:, 16:17]
        b64_sb = bblob_sb[:, 17:18]
        conv_b = [bblob_sb[0:64, 18:19], bblob_sb[0:64, 19:20]]
        cb3_sb = bblob_sb[0:64, 20:21]

        ident = cons.tile([P, P], F32)
        make_identity(nc, ident)

        # persistent intermediates
        q_sb = [cons.tile([P, KT, S], F32R, name=f"q{i}") for i in range(2)]
        kslice_sb = [cons.tile([P, KT, S], F32R, name=f"ksl{i}")
                     for i in range(2)]
        cv_sb = cons.tile([P, HW], F32)
        tv_sb = cons.tile([P, HW], F32)
        strip = cons.tile([P, KT, 4, 4, 2], F32)   # [i, mi, combo, b, h]
        co_sb = cons.tile([P, KT, 4], F32)         # [i, mi, combo]
        co_row = cons.tile([4, S], F32)            # [combo, i]
        gates_sb = cons.tile([16, HW], F32)
        rmax = cons.tile([16, 1], F32)
        negmax = cons.tile([16, 1], F32)
        expacc = cons.tile([16, 1], F32)
        rsum = cons.tile([16, 1], F32)
        gates_n = cons.tile([16, HW], F32R)
        T1 = cons.tile([P, H + 2, W + 2], F32R)
        T2 = cons.tile([P, H + 2, W + 2], F32R)
        T3a = cons.tile([64, H + 2, W + 2], F32R)
        T3b = cons.tile([64, H + 2, W + 2], F32R)
        out_sb = cons.tile([64, H, W], F32)
        zerot = cons.tile([P, H + 2, W + 2], mybir.dt.bfloat16)
        nc.vector.memset(zerot, 0.0)
        nc.vector.tensor_copy(T1, zerot)
        nc.vector.tensor_copy(T2, zerot)
        nc.vector.tensor_copy(T3a, zerot[:64])
        nc.vector.tensor_copy(T3b, zerot[:64])

        kslice_dram = [dram.tile([C, S], F32R, name=f"ksd{i}")
                       for i in range(2)]
        ag_out = [dram.tile([NCORES * C, S], F32R, addr_space="Shared",
                            name=f"ag{i}") for i in range(2)]
        co_dram = dram.tile([4, S], F32)
        co_all = dram.tile([NCORES * 4, S], F32, addr_space="Shared")

        rep = (lambda: tc.For_i(0, time_reps, 1)) if time_reps > 1 else None

        # ---------------- segment 1: projections + values ----------------
        with tc.tile_pool(name="pj", bufs=4, space="PSUM") as pj:
          with rep() if rep else contextlib.nullcontext():
            # k projections first so the AllGathers launch early
            evac_i = 0
            for kk in range(2):
                rhs = (xcq_sb, xtq_sb)[kk]
                for m in range(KT):
                    pq = pj.tile([P, S], F32, tag="pq", name="pq")
                    for kt in range(KT):
                        nc.tensor.matmul(pq, kblob_sb[:, kk, kt,
                                                      P * m:P * (m + 1)],
                                         rhs[:, kt], start=(kt == 0),
                                         stop=(kt == KT - 1))
                    if evac_i % 2 == 0:
                        nc.vector.tensor_scalar_add(kslice_sb[kk][:, m, :], pq,
                                                    bk_sb[kk][:, m:m + 1])
                    else:
                        nc.scalar.activation(kslice_sb[kk][:, m, :], pq,
                                             AF.Identity,
                                             bias=bk_sb[kk][:, m:m + 1])
                    evac_i += 1
                nc.sync.dma_start(
                    kslice_dram[kk].opt().rearrange(KMAJ, p=P), kslice_sb[kk])
            # q projections
            for qi in range(2):
                rhs = (xcq_sb, xtq_sb)[qi]
                for m in range(KT):
                    pq = pj.tile([P, S], F32, tag="pq", name="pq")
                    for kt in range(KT):
                        nc.tensor.matmul(pq, qblob_sb[:, qi, kt,
                                                      P * m:P * (m + 1)],
                                         rhs[:, kt], start=(kt == 0),
                                         stop=(kt == KT - 1))
                    if evac_i % 2 == 0:
                        nc.vector.tensor_scalar_add(q_sb[qi][:, m, :], pq,
                                                    bq_sb[qi][:, m:m + 1])
                    else:
                        nc.scalar.activation(q_sb[qi][:, m, :], pq,
                                             AF.Identity,
                                             bias=bq_sb[qi][:, m:m + 1])
                    evac_i += 1
            # folded 64-channel value projections (duplicated to 128 partitions)
            for vi, (wv, vt) in enumerate(((wcv_sb, cv_sb), (wtv_sb, tv_sb))):
                for nh in range(2):
                    pv = pj.tile([P, 512], F32, tag="pq", name="pv")
                    for kt in range(KT):
                        nc.tensor.matmul(
                            pv, wv[:, kt],
                            (xcb_sb, xtb_sb)[vi][:, kt, 512 * nh:512 * (nh + 1)],
                            start=(kt == 0), stop=(kt == KT - 1))
                    if vi == 0:
                        nc.scalar.activation(vt[:, 512 * nh:512 * (nh + 1)],
                                             pv, AF.Identity, bias=bcv_sb)
                    else:
                        nc.scalar.copy(vt[:, 512 * nh:512 * (nh + 1)], pv)

        # ---------------- k AllGathers ----------------
        for kk in range(2):
            nc.gpsimd.collective_compute(
                "AllGather", mybir.AluOpType.bypass,
                replica_groups=[list(range(NCORES))],
                ins=[kslice_dram[kk].opt()], outs=[ag_out[kk].opt()])

        # ---------------- segment 2: scores + co ----------------
        with tc.tile_pool(name="sc", bufs=6, space="PSUM") as sc, \
             tc.tile_pool(name="fin", bufs=1, space="PSUM") as fin, \
             tc.tile_pool(name="kch", bufs=2) as kch:
          with rep() if rep else contextlib.nullcontext():
            for kk in range(2):
                for cp in range(4):   # chunk pair == one batch b
                    kchunk = kch.tile([P, 2, KT, 512], F32R, tag="kch",
                                      name="kchunk")
                    nc.sync.dma_start(
                        kchunk,
                        ag_out[kk][2 * C * cp:2 * C * (cp + 1), :].rearrange(
                            "(c kt p) n -> p c kt n", c=2, kt=KT, p=P))
                    for h_ in range(2):
                        for mi in range(KT):
                            for qi in range(2):
                                ps = sc.tile([P, 512], F32, tag="ps",
                                             name="ps")
                                for kt in range(KT):
                                    nc.tensor.matmul(
                                        ps,
                                        q_sb[qi][:, kt, P * mi:P * (mi + 1)],
                                        kchunk[:, h_, kt], start=(kt == 0),
                                        stop=(kt == KT - 1))
                                nc.vector.reduce_max(
                                    strip[:, mi, 2 * qi + kk, cp, h_:h_ + 1],
                                    ps, axis=AX)
            # assemble co per m-tile
            for mi in range(KT):
                for qi in range(2):
                    pm = fin.tile([P, 8], F32, tag="pm", name="pm")
                    for kt in range(KT):
                        nc.tensor.matmul(pm,
                                         q_sb[qi][:, kt, P * mi:P * (mi + 1)],
                                         ksums_sb[:, kt], start=(kt == 0),
                                         stop=(kt == KT - 1))
                    mx = cons.tile([P, 2, 4], F32, name="mx", tag="mx")
                    nc.vector.reduce_max(mx, strip[:, mi, 2 * qi:2 * qi + 2],
                                         axis=AX)
                    cmb = cons.tile([P, 2, 4], F32, name="cmb", tag="cmb")
                    nc.vector.tensor_tensor(
                        cmb, mx, pm.rearrange("p (k b) -> p k b", k=2), ADD)
                    nc.vector.reduce_sum(co_sb[:, mi, 2 * qi:2 * qi + 2], cmb,
                                         axis=AX)
                ptr = fin.tile([P, P], F32, tag="ptr", name="ptr")
                nc.tensor.transpose(ptr[:4, :], co_sb[:, mi, :], ident)
                nc.vector.tensor_copy(co_row[:, P * mi:P * (mi + 1)],
                                      ptr[:4, :])
            nc.sync.dma_start(co_dram.opt(), co_row)

        # ---------------- co AllGather ----------------
        nc.gpsimd.collective_compute(
            "AllGather", mybir.AluOpType.bypass,
            replica_groups=[list(range(NCORES))],
            ins=[co_dram.opt()], outs=[co_all.opt()])

        # ---------------- segment 3: gates + fusion convs ----------------
        with tc.tile_pool(name="g", bufs=2, space="PSUM") as g:
          with rep() if rep else contextlib.nullcontext():
            co_view = co_all.opt().rearrange("(b h c) i -> c b h i", b=4,
                                             h=2, c=4)
            for cmb_i in range(4):
                nc.sync.dma_start(
                    gates_sb[4 * cmb_i:4 * (cmb_i + 1), :].rearrange(
                        "p (h i) -> p h i", h=2),
                    co_view[cmb_i])
            nc.vector.reduce_max(rmax, gates_sb, axis=AX)
            nc.vector.tensor_scalar_mul(negmax, rmax, -SCALE)
            expg = cons.tile([16, HW], F32, name="expg")
            nc.scalar.activation(expg, gates_sb, AF.Exp, bias=negmax,
                                 scale=SCALE, accum_out=expacc)
            nc.vector.reciprocal(rsum, expacc)
            nc.vector.tensor_scalar_mul(gates_n, expg, rsum)
            # gate selection + gating, into padded conv inputs
            for ti, (sel, val, T) in enumerate(
                    ((sel1_sb, cv_sb, T1), (sel2_sb, tv_sb, T2))):
                for nh in range(2):
                    pbg = g.tile([P, 512], F32, tag="pbg", name="pbg")
                    nc.tensor.matmul(pbg, sel,
                                     gates_n[:, 512 * nh:512 * (nh + 1)],
                                     start=True, stop=True)
                    reg = T[:, 1 + 16 * nh:17 + 16 * nh, 1:33]
                    nc.vector.tensor_tensor(
                        reg, pbg.rearrange("p (y x) -> p y x", y=16),
                        val[:, 512 * nh:512 * (nh + 1)].rearrange(
                            "p (y x) -> p y x", y=16), MUL)
                    nc.vector.tensor_scalar_add(reg, reg, b64_sb)
            # conv1/conv2: 128-ch input, 64-ch output into T3a/T3b interiors
            for srcT, wi, dstT in ((T1, 0, T3a), (T2, 1, T3b)):
                for cy in range(4):
                    pc = g.tile([64, 8, 32], F32, tag="pc", name="pc")
                    for tap in range(9):
                        dy, dx = tap // 3, tap % 3
                        nc.tensor.matmul(
                            pc, conv_w[wi][:, tap, :],
                            srcT[:, 8 * cy + dy:8 * cy + dy + 8, dx:dx + 32],
                            start=(tap == 0), stop=(tap == 8))
                    nc.scalar.activation(
                        dstT[:, 1 + 8 * cy:9 + 8 * cy, 1:33], pc, AF.Relu,
                        bias=conv_b[wi], scale=1.0)
            # conv3: contraction split into two 64-channel halves
            for cy in range(4):
                pc = g.tile([64, 8, 32], F32, tag="pc", name="pc")
                for hi, (wh, Th) in enumerate(((w3a_sb, T3a), (w3b_sb, T3b))):
                    for tap in range(9):
                        dy, dx = tap // 3, tap % 3
                        nc.tensor.matmul(
                            pc, wh[:, tap, :],
                            Th[:, 8 * cy + dy:8 * cy + dy + 8, dx:dx + 32],
                            start=(hi == 0 and tap == 0),
                            stop=(hi == 1 and tap == 8))
                nc.scalar.activation(out_sb[:, 8 * cy:8 * (cy + 1), :], pc,
                                     AF.Relu, bias=cb3_sb, scale=1.0)
            nc.sync.dma_start(outp_d.ap().rearrange("o (y x) -> o y x", y=H),
                              out_sb)
            if debug:
                nc.sync.dma_start(dbg_co_d.ap(), co_row)
                nc.sync.dma_start(dbg_gates_d.ap(), gates_n.bitcast(F32))
                nc.sync.dma_start(dbg_cv_d.ap(), cv_sb)
                nc.sync.dma_start(
                    dbg_cq_d.ap(),
                    q_sb[0].bitcast(F32))

    nc.compile()
    return nc


# ----------------------------------------------------------------------------
# entry point
# ----------------------------------------------------------------------------

_CACHE = {}


def _get_nc():
    if "nc" not in _CACHE:
        _CACHE["nc"] = build_program()
    return _CACHE["nc"]


def kernel(**inputs) -> np.ndarray:
    nc = _get_nc()
    in_maps = host_prep(inputs)
    res = bass_utils.run_bass_kernel_spmd(nc, in_maps,
                                          core_ids=list(range(NCORES)))
    out = np.empty((B, 64, H, W), np.float32)
    for b in range(B):
        out[b] = res.results[2 * b]["outp"].reshape(64, H, W)
    return out


if __name__ == "__main__":
    # smoke test with random inputs
    rng = np.random.default_rng(0)
    d = {
        "xc": rng.standard_normal((B, C, H, W), np.float32),
        "xt": rng.standard_normal((B, HW, C), np.float32),
    }
    for nm, o in (("q_c", C), ("k_c", C), ("v_c", C), ("q_t", C), ("k_t", C)):
        d[f"W{nm}"] = rng.standard_normal((o, C), np.float32) * 0.02
        d[f"b{nm}"] = np.zeros(o, np.float32)
    d["W512_64"] = rng.standard_normal((64, C), np.float32) * 0.02
    d["b512_64"] = np.zeros(64, np.float32)
    for i in (1, 2, 3):
        d[f"W{i}"] = rng.standard_normal((64, 128, 3, 3), np.float32) * 0.02
        d[f"b{i}"] = np.zeros(64, np.float32)
    out = kernel(**d)
    print("out", out.shape, out.dtype, np.abs(out).max())


# revision 8
# speedup vs baseline: 3.4677x; 3.4677x over previous
"""Trainium2 Bass kernel for nn_MCM_37031208026850.

Strategy (8 NeuronCores, SPMD):
  - Shard the four 4096x512x4096 score GEMMs by query row: core r owns global
    rows [512r, 512(r+1)) (= batch b=r//2, image half r%2).
  - Each core projects its q-slices (cq, tq) and k-slices (ck, tk) locally,
    AllGathers the k projections (512x512 fp32 per core -> 512x4096 full),
    then streams k in 512-column chunks through the PE with q stationary.
  - The mean-over-HW term never touches the score matrix: mean = q @ ksum
    with ksum precomputed on host. Only the max needs the full scores; the
    vector engine max-reduces each PSUM tile as it is produced.
  - Tiny AllGather of the pooled co vectors (4x512 per core); every core
    computes all 16 softmax gates and selects/broadcasts the two gate images
    it needs with host-supplied one-hot matmuls.
  - The 1x1 value convs are folded on host (W512_64 @ Wv_c), computed
    full-batch per core; gating and the three 3x3 fusion convs run
    full-image per core (pair-duplicated), host takes core 2b's output.
  - Score/projection/conv matmuls run in float32r (1 cycle/row on TRN2,
    ~1.4e-4 rel err vs 2.5e-3 for bf16).
"""
import sys
sys.path.insert(0, "/opt/trn_rl_repo")

import numpy as np

import concourse.bass as bass
import concourse.mybir as mybir
import concourse.tile as tile
from concourse import bacc
from concourse import bass_utils
from concourse.masks import make_identity

B, C, H, W = 4, 512, 32, 32
HW = H * W
SCALE = 1.0 / C ** 0.5
NCORES = 8
P = 128
KT = C // P          # 4 k-tiles over channels
S = 512              # q-rows per core
NCH = 8              # global column chunks of 512
F32 = mybir.dt.float32
F32R = mybir.dt.float32r
AX = mybir.AxisListType.X
AF = mybir.ActivationFunctionType
MUL = mybir.AluOpType.mult
ADD = mybir.AluOpType.add


# ----------------------------------------------------------------------------
# host-side preparation
# ----------------------------------------------------------------------------

def host_prep(inputs):
    """Build the 8 per-core input maps from the full problem inputs."""
    xc = np.ascontiguousarray(inputs["xc"], dtype=np.float32)
    xt = np.ascontiguousarray(inputs["xt"], dtype=np.float32)
    f = lambda k: np.ascontiguousarray(inputs[k], dtype=np.float32)
    Wq_c, bq_c = f("Wq_c"), f("bq_c")
    Wk_c, bk_c = f("Wk_c"), f("bk_c")
    Wv_c, bv_c = f("Wv_c"), f("bv_c")
    Wq_t, bq_t = f("Wq_t"), f("bq_t")
    Wk_t, bk_t = f("Wk_t"), f("bk_t")
    W64, b64 = f("W512_64"), f("b512_64")
    W1, b1 = f("W1"), f("b1")
    W2, b2 = f("W2"), f("b2")
    W3, b3 = f("W3"), f("b3")

    xcG = np.ascontiguousarray(
        xc.reshape(B, C, HW).transpose(1, 0, 2).reshape(C, B * HW))
    xtT = np.ascontiguousarray(
        xt.transpose(2, 0, 1).reshape(C, B * HW))

    # ksum[c, kk*4+b]: column-sums of the k matrices per batch, /HW (mean),
    # computed from input sums so the score matrix is never needed.
    xc_sum = xc.sum(axis=(2, 3))                      # (B, C)
    xt_sum = xt.sum(axis=1)                           # (B, C)
    ks_ck = (Wk_c @ xc_sum.T + HW * bk_c[:, None]) / HW    # (C, B)
    ks_tk = (Wk_t @ xt_sum.T + HW * bk_t[:, None]) / HW    # (C, B)
    ksums = np.concatenate([ks_ck, ks_tk], axis=1)    # (C, 8) [kk*4+b]

    Wcv = W64 @ Wv_c                                  # (64, C)
    bcv = W64 @ bv_c                                  # (64,)
    wcv64 = np.ascontiguousarray(np.concatenate([Wcv, Wcv], axis=0).T)   # (C,128)
    wtv64 = np.ascontiguousarray(np.concatenate([W64, W64], axis=0).T)   # (C,128)
    bcv64 = np.concatenate([bcv, bcv]).reshape(P, 1)
    b64dup = np.concatenate([b64, b64]).reshape(P, 1)

    def kmaj(a):
        # (C, n) -> (P, KT*n) k-major layout matching SBUF tiles
        n = a.shape[1]
        return a.reshape(KT, P, n).transpose(1, 0, 2).reshape(P, KT * n)

    # blob_k: k-projection weights (needed first)
    blob_k = np.hstack([kmaj(Wk_c.T), kmaj(Wk_t.T)])                  # (P, 4096)
    # blob_q: q-projection weights
    blob_q = np.hstack([kmaj(Wq_c.T), kmaj(Wq_t.T)])                  # (P, 4096)
    w3t_full = W3.transpose(1, 2, 3, 0).reshape(P, 9 * 64)
    pad = np.zeros((P - 64, 9 * 64), np.float32)
    # blob_v: value weights + ksums + sels + conv weights
    sel_pad = np.zeros((P, 2 * P), np.float32)   # filled per-core later
    blob_v_shared = np.hstack([
        kmaj(wcv64), kmaj(wtv64), kmaj(ksums),
        W1.transpose(1, 2, 3, 0).reshape(P, 9 * 64),
        W2.transpose(1, 2, 3, 0).reshape(P, 9 * 64),
        np.vstack([w3t_full[:64], pad]),
        np.vstack([w3t_full[64:], pad]),
    ])                                                                # (P, ...)
    # blob_b (fp32): all biases column-packed
    cpad = np.zeros((P - 64, 1), np.float32)
    blob_b = np.hstack([
        bq_c.reshape(KT, P).T, bk_c.reshape(KT, P).T,
        bq_t.reshape(KT, P).T, bk_t.reshape(KT, P).T,
        bcv64, b64dup,
        np.vstack([b1.reshape(64, 1), cpad]),
        np.vstack([b2.reshape(64, 1), cpad]),
        np.vstack([b3.reshape(64, 1), cpad]),
    ]).astype(np.float32)                                             # (P, 19)
    shared = {
        "blob_k": np.ascontiguousarray(blob_k, dtype=np.float32),
        "blob_q": np.ascontiguousarray(blob_q, dtype=np.float32),
        "blob_b": np.ascontiguousarray(blob_b, dtype=np.float32),
    }

    in_maps = []
    for r in range(NCORES):
        myb = r // 2
        cols = slice(S * r, S * (r + 1))
        bcols = slice(HW * myb, HW * (myb + 1))
        sel1 = np.zeros((16, P), np.float32)   # T1 = [c_co; ct_co] combos 0,1
        sel2 = np.zeros((16, P), np.float32)   # T2 = [t_co; tc_co] combos 3,2
        for p in range(P):
            sel1[(0 if p < 64 else 1) * 4 + myb, p] = 1.0
            sel2[(3 if p < 64 else 2) * 4 + myb, p] = 1.0
        m = dict(shared)
        m["xcq"] = np.ascontiguousarray(xcG[:, cols])
        m["xtq"] = np.ascontiguousarray(xtT[:, cols])
        m["xcb"] = np.ascontiguousarray(xcG[:, bcols])
        m["xtb"] = np.ascontiguousarray(xtT[:, bcols])
        m["blob_v"] = np.ascontiguousarray(
            np.hstack([blob_v_shared,
                       np.vstack([sel1, np.zeros((P - 16, P), np.float32)]),
                       np.vstack([sel2, np.zeros((P - 16, P), np.float32)])]),
            dtype=np.float32)
        in_maps.append(m)
    return in_maps


# ----------------------------------------------------------------------------
# device program
# ----------------------------------------------------------------------------

def build_program(time_reps: int = 1, debug: bool = False):
    """Build + bacc-compile the SPMD Bass program.

    time_reps > 1 wraps the three compute segments in For_i loops (collectives
    stay outside) so wall-clock deltas between different reps counts measure
    pure per-iteration compute time.
    """
    import contextlib
    nc = bacc.Bacc("TRN2", target_bir_lowering=False, debug=False,
                   num_devices=NCORES)

    def din(name, shape, dtype=F32R):
        return nc.dram_tensor(name, list(shape), dtype, kind="ExternalInput")

    xcq_d = din("xcq", (C, S)); xtq_d = din("xtq", (C, S))
    xcb_d = din("xcb", (C, HW)); xtb_d = din("xtb", (C, HW))
    blob_k_d = din("blob_k", (P, 2 * KT * 512))
    blob_q_d = din("blob_q", (P, 2 * KT * 512))
    VBLOB = 512 + 512 + 32 + 576 + 576 + 576 + 576 + 128 + 128
    blob_v_d = din("blob_v", (P, VBLOB))
    blob_b_d = din("blob_b", (P, 21), F32)

    outp_d = nc.dram_tensor("outp", [64, HW], F32, kind="ExternalOutput")
    if debug:
        dbg_co_d = nc.dram_tensor("dbg_co", [4, S], F32, kind="ExternalOutput")
        dbg_gates_d = nc.dram_tensor("dbg_gates", [16, HW], F32,
                                     kind="ExternalOutput")
        dbg_cv_d = nc.dram_tensor("dbg_cv", [P, HW], F32, kind="ExternalOutput")
        dbg_cq_d = nc.dram_tensor("dbg_cq", [P, KT, S], F32,
                                  kind="ExternalOutput")

    KMAJ = "(kt p) n -> p kt n"

    with tile.TileContext(nc) as tc:
      with tc.tile_pool(name="consts", bufs=1) as cons, \
           tc.tile_pool(name="dram", bufs=1, space="DRAM") as dram:
        # ---------------- static loads (phase-ordered) ----------------
        xcq_sb = cons.tile([P, KT, S], F32R)
        xtq_sb = cons.tile([P, KT, S], F32R)
        nc.sync.dma_start(xcq_sb, xcq_d.ap().rearrange(KMAJ, p=P))
        nc.sync.dma_start(xtq_sb, xtq_d.ap().rearrange(KMAJ, p=P))
        kblob_sb = cons.tile([P, 2, KT, 512], F32R)
        nc.sync.dma_start(
            kblob_sb, blob_k_d.ap().rearrange("p (k kt n) -> p k kt n",
                                              k=2, kt=KT))
        qblob_sb = cons.tile([P, 2, KT, 512], F32R)
        nc.sync.dma_start(
            qblob_sb, blob_q_d.ap().rearrange("p (k kt n) -> p k kt n",
                                              k=2, kt=KT))
        vblob_sb = cons.tile([P, VBLOB], F32R)
        nc.sync.dma_start(vblob_sb, blob_v_d.ap())
        bblob_sb = cons.tile([P, 21], F32)
        nc.sync.dma_start(bblob_sb, blob_b_d.ap())
        xcb_sb = cons.tile([P, KT, HW], F32R)
        xtb_sb = cons.tile([P, KT, HW], F32R)
        nc.sync.dma_start(xcb_sb, xcb_d.ap().rearrange(KMAJ, p=P))
        nc.sync.dma_start(xtb_sb, xtb_d.ap().rearrange(KMAJ, p=P))

        # blob views
        def vsl(lo, n):
            return vblob_sb[:, lo:lo + n]

        wcv_sb = vsl(0, 512).rearrange("p (kt n) -> p kt n", kt=KT)
        wtv_sb = vsl(512, 512).rearrange("p (kt n) -> p kt n", kt=KT)
        ksums_sb = vsl(1024, 32).rearrange("p (kt n) -> p kt n", kt=KT)
        w1t_sb = vsl(1056, 576).rearrange("p (t n) -> p t n", t=9)
        w2t_sb = vsl(1632, 576).rearrange("p (t n) -> p t n", t=9)
        w3a_sb = vblob_sb[0:64, 2208:2784].rearrange("p (t n) -> p t n", t=9)
        w3b_sb = vblob_sb[0:64, 2784:3360].rearrange("p (t n) -> p t n", t=9)
        sel1_sb = vblob_sb[0:16, 3360:3488]
        sel2_sb = vblob_sb[0:16, 3488:3616]
        conv_w = [w1t_sb, w2t_sb]
        bq_sb = [bblob_sb[:, 0:4], bblob_sb[:, 8:12]]
        bk_sb = [bblob_sb[:, 4:8], bblob_sb[:, 12:16]]
        bcv_sb = bblob_sb[:, 16:17]
        b64_sb = bblob_sb[:, 17:18]
        conv_b = [bblob_sb[0:64, 18:19], bblob_sb[0:64, 19:20]]
        cb3_sb = bblob_sb[0:64, 20:21]

        ident = cons.tile([P, P], F32)
        make_identity(nc, ident)

        # persistent intermediates
        q_sb = [cons.tile([P, KT, S], F32R, name=f"q{i}") for i in range(2)]
        kslice_sb = [cons.tile([P, KT, S], F32R, name=f"ksl{i}")
                     for i in range(2)]
        cv_sb = cons.tile([P, HW], F32)
        tv_sb = cons.tile([P, HW], F32)
        strip = cons.tile([P, KT, 4, 4, 2], F32)   # [i, mi, combo, b, h]
        co_sb = cons.tile([P, KT, 4], F32)         # [i, mi, combo]
        co_row = cons.tile([4, S], F32)            # [combo, i]
        gates_sb = cons.tile([16, HW], F32)
        rmax = cons.tile([16, 1], F32)
        negmax = cons.tile([16, 1], F32)
        expacc = cons.tile([16, 1], F32)
        rsum = cons.tile([16, 1], F32)
        gates_n = cons.tile([16, HW], F32R)
        T1 = cons.tile([P, H + 2, W + 2], F32R)
        T2 = cons.tile([P, H + 2, W + 2], F32R)
        T3a = cons.tile([64, H + 2, W + 2], F32R)
        T3b = cons.tile([64, H + 2, W + 2], F32R)
        out_sb = cons.tile([64, H, W], F32)
        zerot = cons.tile([P, H + 2, W + 2], mybir.dt.bfloat16)
        nc.vector.memset(zerot, 0.0)
        nc.vector.tensor_copy(T1, zerot)
        nc.vector.tensor_copy(T2, zerot)
        nc.vector.tensor_copy(T3a, zerot[:64])
        nc.vector.tensor_copy(T3b, zerot[:64])

        kslice_dram = [dram.tile([C, S], F32R, name=f"ksd{i}")
                       for i in range(2)]
        ag_out = [dram.tile([NCORES * C, S], F32R, addr_space="Shared",
                            name=f"ag{i}") for i in range(2)]
        co_dram = dram.tile([4, S], F32)
        co_all = dram.tile([NCORES * 4, S], F32, addr_space="Shared")

        rep = (lambda: tc.For_i(0, time_reps, 1)) if time_reps > 1 else None

        # ---------------- segment 1: projections + values ----------------
        with tc.tile_pool(name="pj", bufs=4, space="PSUM") as pj:
          with rep() if rep else contextlib.nullcontext():
            # k projections first so the AllGathers launch early
            evac_i = 0
            for kk in range(2):
                rhs = (xcq_sb, xtq_sb)[kk]
                for m in range(KT):
                    pq = pj.tile([P, S], F32, tag="pq", name="pq")
                    for kt in range(KT):
                        nc.tensor.matmul(pq, kblob_sb[:, kk, kt,
                                                      P * m:P * (m + 1)],
                                         rhs[:, kt], start=(kt == 0),
                                         stop=(kt == KT - 1))
                    if evac_i % 2 == 0:
                        nc.vector.tensor_scalar_add(kslice_sb[kk][:, m, :], pq,
                                                    bk_sb[kk][:, m:m + 1])
                    else:
                        nc.scalar.activation(kslice_sb[kk][:, m, :], pq,
                                             AF.Identity,
                                             bias=bk_sb[kk][:, m:m + 1])
                    evac_i += 1
                nc.sync.dma_start(
                    kslice_dram[kk].opt().rearrange(KMAJ, p=P), kslice_sb[kk])
            # q projections
            for qi in range(2):
                rhs = (xcq_sb, xtq_sb)[qi]
                for m in range(KT):
                    pq = pj.tile([P, S], F32, tag="pq", name="pq")
                    for kt in range(KT):
                        nc.tensor.matmul(pq, qblob_sb[:, qi, kt,
                                                      P * m:P * (m + 1)],
                                         rhs[:, kt], start=(kt == 0),
                                         stop=(kt == KT - 1))
                    if evac_i % 2 == 0:
                        nc.vector.tensor_scalar_add(q_sb[qi][:, m, :], pq,
                                                    bq_sb[qi][:, m:m + 1])
                    else:
                        nc.scalar.activation(q_sb[qi][:, m, :], pq,
                                             AF.Identity,
                                             bias=bq_sb[qi][:, m:m + 1])
                    evac_i += 1

        # ---------------- k AllGathers ----------------
        for kk in range(2):
            nc.gpsimd.collective_compute(
                "AllGather", mybir.AluOpType.bypass,
                replica_groups=[list(range(NCORES))],
                ins=[kslice_dram[kk].opt()], outs=[ag_out[kk].opt()])

        # ---------------- segment 2: scores + co ----------------
        with tc.tile_pool(name="sc", bufs=6, space="PSUM") as sc, \
             tc.tile_pool(name="fin", bufs=1, space="PSUM") as fin, \
             tc.tile_pool(name="kch", bufs=2) as kch:
          with rep() if rep else contextlib.nullcontext():
            for kk in range(2):
                for cp in range(4):   # chunk pair == one batch b
                    kchunk = kch.tile([P, 2, KT, 512], F32R, tag="kch",
                                      name="kchunk")
                    nc.sync.dma_start(
                        kchunk,
                        ag_out[kk][2 * C * cp:2 * C * (cp + 1), :].rearrange(
                            "(c kt p) n -> p c kt n", c=2, kt=KT, p=P))
                    for h_ in range(2):
                        for mi in range(KT):
                            for qi in range(2):
                                ps = sc.tile([P, 512], F32, tag="ps",
                                             name="ps")
                                for kt in range(KT):
                                    nc.tensor.matmul(
                                        ps,
                                        q_sb[qi][:, kt, P * mi:P * (mi + 1)],
                                        kchunk[:, h_, kt], start=(kt == 0),
                                        stop=(kt == KT - 1))
                                nc.vector.reduce_max(
                                    strip[:, mi, 2 * qi + kk, cp, h_:h_ + 1],
                                    ps, axis=AX)
            # assemble co per m-tile
            for mi in range(KT):
                for qi in range(2):
                    pm = fin.tile([P, 8], F32, tag="pm", name="pm")
                    for kt in range(KT):
                        nc.tensor.matmul(pm,
                                         q_sb[qi][:, kt, P * mi:P * (mi + 1)],
                                         ksums_sb[:, kt], start=(kt == 0),
                                         stop=(kt == KT - 1))
                    mx = cons.tile([P, 2, 4], F32, name="mx", tag="mx")
                    nc.vector.reduce_max(mx, strip[:, mi, 2 * qi:2 * qi + 2],
                                         axis=AX)
                    cmb = cons.tile([P, 2, 4], F32, name="cmb", tag="cmb")
                    nc.vector.tensor_tensor(
                        cmb, mx, pm.rearrange("p (k b) -> p k b", k=2), ADD)
                    nc.vector.reduce_sum(co_sb[:, mi, 2 * qi:2 * qi + 2], cmb,
                                         axis=AX)
                ptr = fin.tile([P, P], F32, tag="ptr", name="ptr")
                nc.tensor.transpose(ptr[:4, :], co_sb[:, mi, :], ident)
                nc.vector.tensor_copy(co_row[:, P * mi:P * (mi + 1)],
                                      ptr[:4, :])
            nc.sync.dma_start(co_dram.opt(), co_row)

        # ---------------- co AllGather ----------------
        nc.gpsimd.collective_compute(
            "AllGather", mybir.AluOpType.bypass,
            replica_groups=[list(range(NCORES))],
            ins=[co_dram.opt()], outs=[co_all.opt()])

        # ---------------- segment 3: gates + fusion convs ----------------
        with tc.tile_pool(name="g", bufs=2, space="PSUM") as g:
          with rep() if rep else contextlib.nullcontext():
            co_view = co_all.opt().rearrange("(b h c) i -> c b h i", b=4,
                                             h=2, c=4)
            for cmb_i in range(4):
                nc.sync.dma_start(
                    gates_sb[4 * cmb_i:4 * (cmb_i + 1), :].rearrange(
                        "p (h i) -> p h i", h=2),
                    co_view[cmb_i])
            # folded 64-channel value projections (fill the co-AG bubble)
            for vi, (wv, vt) in enumerate(((wcv_sb, cv_sb), (wtv_sb, tv_sb))):
                for nh in range(2):
                    pv = g.tile([P, 512], F32, tag="pv", name="pv")
                    for kt in range(KT):
                        nc.tensor.matmul(
                            pv, wv[:, kt],
                            (xcb_sb, xtb_sb)[vi][:, kt, 512 * nh:512 * (nh + 1)],
                            start=(kt == 0), stop=(kt == KT - 1))
                    if vi == 0:
                        nc.scalar.activation(vt[:, 512 * nh:512 * (nh + 1)],
                                             pv, AF.Identity, bias=bcv_sb)
                    else:
                        nc.scalar.copy(vt[:, 512 * nh:512 * (nh + 1)], pv)
            nc.vector.reduce_max(rmax, gates_sb, axis=AX)
            nc.vector.tensor_scalar_mul(negmax, rmax, -SCALE)
            expg = cons.tile([16, HW], F32, name="expg")
            nc.scalar.activation(expg, gates_sb, AF.Exp, bias=negmax,
                                 scale=SCALE, accum_out=expacc)
            nc.vector.reciprocal(rsum, expacc)
            nc.vector.tensor_scalar_mul(gates_n, expg, rsum)
            # gate selection + gating, into padded conv inputs
            for ti, (sel, val, T) in enumerate(
                    ((sel1_sb, cv_sb, T1), (sel2_sb, tv_sb, T2))):
                for nh in range(2):
                    pbg = g.tile([P, 512], F32, tag="pbg", name="pbg")
                    nc.tensor.matmul(pbg, sel,
                                     gates_n[:, 512 * nh:512 * (nh + 1)],
                                     start=True, stop=True)
                    reg = T[:, 1 + 16 * nh:17 + 16 * nh, 1:33]
                    nc.vector.tensor_tensor(
                        reg, pbg.rearrange("p (y x) -> p y x", y=16),
                        val[:, 512 * nh:512 * (nh + 1)].rearrange(
                            "p (y x) -> p y x", y=16), MUL)
                    nc.vector.tensor_scalar_add(reg, reg, b64_sb)
            # conv1/conv2: 128-ch input, 64-ch output into T3a/T3b interiors
            for srcT, wi, dstT in ((T1, 0, T3a), (T2, 1, T3b)):
                for cy in range(4):
                    pc = g.tile([64, 8, 32], F32, tag="pc", name="pc")
                    for tap in range(9):
                        dy, dx = tap // 3, tap % 3
                        nc.tensor.matmul(
                            pc, conv_w[wi][:, tap, :],
                            srcT[:, 8 * cy + dy:8 * cy + dy + 8, dx:dx + 32],
                            start=(tap == 0), stop=(tap == 8))
                    nc.scalar.activation(
                        dstT[:, 1 + 8 * cy:9 + 8 * cy, 1:33], pc, AF.Relu,
                        bias=conv_b[wi], scale=1.0)
            # conv3: contraction split into two 64-channel halves
            for cy in range(4):
                pc = g.tile([64, 8, 32], F32, tag="pc", name="pc")
                for hi, (wh, Th) in enumerate(((w3a_sb, T3a), (w3b_sb, T3b))):
                    for tap in range(9):
                        dy, dx = tap // 3, tap % 3
                        nc.tensor.matmul(
                            pc, wh[:, tap, :],
                            Th[:, 8 * cy + dy:8 * cy + dy + 8, dx:dx + 32],
                            start=(hi == 0 and tap == 0),
                            stop=(hi == 1 and tap == 8))
                nc.scalar.activation(out_sb[:, 8 * cy:8 * (cy + 1), :], pc,
                                     AF.Relu, bias=cb3_sb, scale=1.0)
            nc.sync.dma_start(outp_d.ap().rearrange("o (y x) -> o y x", y=H),
                              out_sb)
            if debug:
                nc.sync.dma_start(dbg_co_d.ap(), co_row)
                nc.sync.dma_start(dbg_gates_d.ap(), gates_n.bitcast(F32))
                nc.sync.dma_start(dbg_cv_d.ap(), cv_sb)
                nc.sync.dma_start(
                    dbg_cq_d.ap(),
                    q_sb[0].bitcast(F32))

    nc.compile()
    return nc


# ----------------------------------------------------------------------------
# entry point
# ----------------------------------------------------------------------------

_CACHE = {}


def _get_nc():
    if "nc" not in _CACHE:
        _CACHE["nc"] = build_program()
    return _CACHE["nc"]


def kernel(**inputs) -> np.ndarray:
    nc = _get_nc()
    in_maps = host_prep(inputs)
    res = bass_utils.run_bass_kernel_spmd(nc, in_maps,
                                          core_ids=list(range(NCORES)))
    out = np.empty((B, 64, H, W), np.float32)
    for b in range(B):
        out[b] = res.results[2 * b]["outp"].reshape(64, H, W)
    return out


if __name__ == "__main__":
    # smoke test with random inputs
    rng = np.random.default_rng(0)
    d = {
        "xc": rng.standard_normal((B, C, H, W), np.float32),
        "xt": rng.standard_normal((B, HW, C), np.float32),
    }
    for nm, o in (("q_c", C), ("k_c", C), ("v_c", C), ("q_t", C), ("k_t", C)):
        d[f"W{nm}"] = rng.standard_normal((o, C), np.float32) * 0.02
        d[f"b{nm}"] = np.zeros(o, np.float32)
    d["W512_64"] = rng.standard_normal((64, C), np.float32) * 0.02
    d["b512_64"] = np.zeros(64, np.float32)
    for i in (1, 2, 3):
        d[f"W{i}"] = rng.standard_normal((64, 128, 3, 3), np.float32) * 0.02
        d[f"b{i}"] = np.zeros(64, np.float32)
    out = kernel(**d)
    print("out", out.shape, out.dtype, np.abs(out).max())
